# revision 32
# baseline (speedup 1.0000x reference)
"""Bass/Trainium2 kernel for nn_Attention_21354577395789.

Reference computation (B=16, S=2048, H=1024, D=2H=2048):
    h      = broadcast(hidden[1, 2H]) -> [B, S, 2H]
    cat    = concat([h, enc], -1)                    [B, S, 4H]
    energy = tanh(cat @ attn_w.T + attn_b)           [B, S, H]
    scores = energy @ v_w.T                          [B, S, 1]
    attn   = softmax(scores, axis=1)
    ctx    = attn^T @ enc                            [B, 1, 2H]

Algebraic simplifications:
  * attn_w = [W_h | W_e] along its 4H input dim, so
    cat @ attn_w.T = hidden @ W_h.T + enc @ W_e.T and
    c = hidden @ W_h.T + attn_b is a single [H] vector shared by every
    (b, s). c is computed on the HOST (it is tiny) and uploaded.
  * scores are O(1) in magnitude, so softmax needs no max subtraction:
    u = exp(s) streamed per 512-chunk of S; the device emits
    per-chunk unnormalized context partials and per-chunk exp-sums,
    and the final (sum over chunks) / (sum of exp) happens on host.

Engine placement per 512-column chunk of S:
  PE    : energy matmuls (the only O(S*D*H) work) + v-reduction
  ACT   : tanh(+bias), exp(+chunk sum)
  GpSimd: broadcast exp-weights row across 128 partitions
  Vector: fused multiply+sum of resident encT tiles against the
          broadcast weights (context partials)
enc is loaded from DRAM exactly once (d-major layout only).

Distribution: data-parallel over B across 8 NeuronCores (2 batches per
core), no collectives. Compute in bf16 (fp32 PSUM accumulation).
"""

import os

import numpy as np
import ml_dtypes

B, S, H = 16, 2048, 1024
D = 2 * H          # 2048, encoder feature dim / contraction dim of W_e
N_CORES = 8
BPC = B // N_CORES  # batches per core = 2
NT = 512           # t-chunk (moving-dim) size
KT = D // 128      # 16 k-tiles over the contraction dim d
JT = H // 128      # 8 j-tiles over the energy dim
TBLK = S // NT     # 4 t-chunks per batch

# Per-batch chunk widths. The LAST batch ends with two narrow chunks so
# the post-energy softmax+context tail (which cannot overlap anything)
# is as short as possible.
CHUNKS = [[512] * 4 for _ in range(BPC)]
CHUNKS[BPC - 1] = [512, 512, 512, 256, 128, 128]
NCH = max(len(c) for c in CHUNKS)

BF16 = ml_dtypes.bfloat16

_cache = {}


def _build():
    import concourse.bacc as bacc
    import concourse.tile as tile
    from concourse import mybir

    nc = bacc.Bacc("TRN2", target_bir_lowering=False, debug=False)
    dt = mybir.dt

    # encT2[b, p, kk*S + t] = enc[b, t, kk*128 + p]
    encT2 = nc.declare_dram_parameter(
        "encT2", [BPC, 128, KT * S], dt.bfloat16, isOutput=False
    )
    # w_j2[p, kk*H + jj*128 + j] = w_eT[kk*128 + p, jj*128 + j]
    # (identical layout to the SBUF-resident copy: one contiguous DMA per kk)
    w_j2 = nc.declare_dram_parameter(
        "w_j2", [128, KT * H], dt.bfloat16, isOutput=False
    )
    c_cols_d = nc.declare_dram_parameter("c_cols", [128, JT], dt.float32, isOutput=False)
    v_cols_d = nc.declare_dram_parameter("v_cols", [128, JT], dt.bfloat16, isOutput=False)
    out_part = nc.declare_dram_parameter(
        "out_part", [BPC, 128, NCH * KT], dt.float32, isOutput=True
    )
    out_sums = nc.declare_dram_parameter(
        "out_sums", [BPC, 1, NCH], dt.float32, isOutput=True
    )

    AF = mybir.ActivationFunctionType
    OP = mybir.AluOpType

    with tile.TileContext(nc) as tc:
        with (
            tc.tile_pool(name="weights", bufs=1) as wpool,
            tc.tile_pool(name="enc", bufs=4) as encpool,
            tc.tile_pool(name="energy", bufs=2) as epool,
            tc.tile_pool(name="usoft", bufs=2) as upool,
            tc.tile_pool(name="scr", bufs=2) as scrpool,
            tc.tile_pool(name="perb", bufs=2) as bpool,
            tc.tile_pool(name="psum_e", bufs=6, space="PSUM") as pe_pool,
            tc.tile_pool(name="psum_s", bufs=2, space="PSUM") as ps_pool,
        ):
            # ---- resident weights/constants -----------------------------
            # kk-major layout (kk*H + jj*128 + j): each per-kk DMA writes
            # one contiguous 2KB run per partition (256B runs fragment DMA
            # into slow small packets), and the jj=0 energy matmuls can
            # start as soon as the first kk slices land.
            w_sb = wpool.tile([128, KT * H], dt.bfloat16, tag="w")
            c_sb = wpool.tile([128, JT], dt.float32, tag="c")
            v_sb = wpool.tile([128, JT], dt.bfloat16, tag="v")

            def w_stat(kk, jj):
                o = kk * H + jj * 128
                return w_sb[:, o : o + 128]

            def dma_w(kk):
                nc.sync.dma_start(
                    w_sb[:, kk * H : (kk + 1) * H],
                    w_j2.ap()[:, kk * H : (kk + 1) * H],
                )

            enc_tiles = {}
            offs = [
                [sum(CHUNKS[b][:ci]) for ci in range(len(CHUNKS[b]))]
                for b in range(BPC)
            ]

            def dma_enc(b, ci, split=False):
                w = CHUNKS[b][ci]
                t0 = offs[b][ci]
                enc_t = encpool.tile(
                    [128, KT * NT], dt.bfloat16, tag="enc", name=f"enc{b}_{ci}"
                )
                src = encT2.ap()[b]
                if split:
                    for k0 in range(0, KT, 4):
                        nc.sync.dma_start(
                            enc_t[:, k0 * w : (k0 + 4) * w].rearrange(
                                "p (kk t) -> p kk t", kk=4, t=w
                            ),
                            src.rearrange("p (kk t) -> p kk t", kk=KT, t=S)[
                                :, k0 : k0 + 4, t0 : t0 + w
                            ],
                        )
                else:
                    nc.sync.dma_start(
                        enc_t[:, : KT * w].rearrange("p (kk t) -> p kk t", kk=KT, t=w),
                        src.rearrange("p (kk t) -> p kk t", kk=KT, t=S)[
                            :, :, t0 : t0 + w
                        ],
                    )
                enc_tiles[(b, ci)] = enc_t

            # startup: interleave per-kk stationary slices with the first
            # enc chunk's 4-kk groups so the jj=0 matmuls stream behind
            # the DMA arrivals kk by kk.
            # constants first: the very first tanh needs c_sb, and these
            # 4.5KB cost nothing in bandwidth
            nc.sync.dma_start(c_sb[:], c_cols_d.ap()[:])
            nc.sync.dma_start(v_sb[:], v_cols_d.ap()[:])
            first = encpool.tile([128, KT * NT], dt.bfloat16, tag="enc", name="enc0_0")
            src0 = encT2.ap()[0].rearrange("p (kk t) -> p kk t", kk=KT, t=S)
            w00 = CHUNKS[0][0]
            for k0 in range(0, KT, 4):
                dma_w(k0)
                dma_w(k0 + 1)
                nc.sync.dma_start(
                    first[:, k0 * w00 : (k0 + 4) * w00].rearrange(
                        "p (kk t) -> p kk t", kk=4, t=w00
                    ),
                    src0[:, k0 : k0 + 4, 0:w00],
                )
                dma_w(k0 + 2)
                dma_w(k0 + 3)
            enc_tiles[(0, 0)] = first
            dma_enc(0, 1)

            sums_t = {}
            part_t = {}
            for b in range(BPC):
                sums_t[b] = bpool.tile(
                    [1, NCH], dt.float32, tag="sums", name=f"sums{b}"
                )
                # per-chunk context partials: part[:, ci*KT + kk]
                part_t[b] = bpool.tile(
                    [128, NCH * KT], dt.float32, tag="part", name=f"part{b}"
                )

            all_chunks = [
                (b, ci) for b in range(BPC) for ci in range(len(CHUNKS[b]))
            ]

            # The v-reduction matmuls of chunk c (which wait on chunk c's
            # tanh outputs) are deferred into chunk c+1's first energy
            # block so the PE never stalls at a chunk boundary, and run as
            # one contiguous block so they break the energy weights'
            # stationary-preload chain only twice per chunk.
            carry = None  # (b, ci, w, s_ps, e_all, enc_t)

            def finish_chunk(b, ci, w, s_ps, enc_t):
                # streaming softmax chunk: u = exp(s), chunk sum
                u_row = upool.tile([1, NT], dt.float16, tag="urow")
                nc.scalar.activation(
                    u_row[:, :w], s_ps[:, :w], AF.Exp,
                    accum_out=sums_t[b][0:1, ci : ci + 1],
                )
                u_bc = upool.tile([128, NT], dt.float16, tag="ubc")
                nc.gpsimd.partition_broadcast(u_bc[:, :w], u_row[:, :w])
                # context partials: part[:, ci*KT+kk] = sum_t u_t * encT[d, t]
                for kk in range(KT):
                    scratch = scrpool.tile([128, NT], dt.bfloat16, tag="scr")
                    nc.vector.scalar_tensor_tensor(
                        out=scratch[:, :w],
                        in0=enc_t[:, kk * w : (kk + 1) * w],
                        scalar=1.0,
                        in1=u_bc[:, :w],
                        op0=OP.mult,
                        op1=OP.mult,
                        accum_out=part_t[b][:, ci * KT + kk : ci * KT + kk + 1],
                    )
                nch = len(CHUNKS[b])
                if ci == nch - 1:
                    nc.sync.dma_start(
                        out_part.ap()[b][:, : nch * KT], part_t[b][:, : nch * KT]
                    )
                    nc.sync.dma_start(
                        out_sums.ap()[b][:, :nch], sums_t[b][0:1, :nch]
                    )

            for b, ci in all_chunks:
                w = CHUNKS[b][ci]
                if (b, ci) not in enc_tiles:
                    dma_enc(b, ci)
                enc_t = enc_tiles.pop((b, ci))
                # prefetch next chunk right away
                nch = len(CHUNKS[b])
                nb, nci = (b, ci + 1) if ci + 1 < nch else (b + 1, 0)
                if nb < BPC and (nb, nci) not in enc_tiles:
                    dma_enc(nb, nci)

                s_ps = ps_pool.tile(
                    [1, NT], dt.float32, tag="sps", name=f"sps{b}_{ci}"
                )
                e_all = epool.tile(
                    [128, JT * NT], dt.bfloat16, tag="eall", name=f"eall{b}_{ci}"
                )
                for jj in range(JT):
                    e_ps = pe_pool.tile([128, NT], dt.float32, tag="eps")
                    for kk in range(KT):
                        nc.tensor.matmul(
                            e_ps[:, :w],
                            w_stat(kk, jj),
                            enc_t[:, kk * w : (kk + 1) * w],
                            start=(kk == 0),
                            stop=(kk == KT - 1),
                        )
                    if jj == 0 and carry is not None:
                        pb, pci, pw, ps_ps, pe_all, penc_t = carry
                        for j in range(JT):
                            nc.tensor.matmul(
                                ps_ps[:, :pw], v_sb[:, j : j + 1],
                                pe_all[:, j * NT : j * NT + pw],
                                start=(j == 0), stop=(j == JT - 1),
                            )
                        finish_chunk(pb, pci, pw, ps_ps, penc_t)
                        carry = None
                    nc.scalar.activation(
                        e_all[:, jj * NT : jj * NT + w], e_ps[:, :w], AF.Tanh,
                        bias=c_sb[:, jj : jj + 1],
                    )
                carry = (b, ci, w, s_ps, e_all, enc_t)

            # drain the final chunk
            pb, pci, pw, ps_ps, pe_all, penc_t = carry
            for j in range(JT):
                nc.tensor.matmul(
                    ps_ps[:, :pw], v_sb[:, j : j + 1],
                    pe_all[:, j * NT : j * NT + pw],
                    start=(j == 0), stop=(j == JT - 1),
                )
            finish_chunk(pb, pci, pw, ps_ps, penc_t)

    nc.compile()
    return nc


def _get_nc():
    if "nc" not in _cache:
        import time

        t0 = time.time()
        _cache["nc"] = _build()
        if os.environ.get("KERNEL_TRACE"):
            print(f"[kernel] bass build+compile: {time.time() - t0:.1f} s")
    return _cache["nc"]


def kernel(hidden, encoder_outputs, attn_w, attn_b, v_w):
    from concourse.bass_utils import run_bass_kernel_spmd

    nc = _get_nc()

    hidden = np.asarray(hidden, dtype=np.float32)
    enc = np.asarray(encoder_outputs, dtype=np.float32)
    attn_w = np.asarray(attn_w, dtype=np.float32)
    attn_b = np.asarray(attn_b, dtype=np.float32)
    v_w = np.asarray(v_w, dtype=np.float32)

    w_eT = np.ascontiguousarray(attn_w[:, D:].T)                 # [D, H]
    # (kk, p, jh) -> (p, kk, jh): same layout as the SBUF-resident copy
    w_j2 = np.ascontiguousarray(
        w_eT.reshape(KT, 128, H).transpose(1, 0, 2).reshape(128, KT * H)
    ).astype(BF16)
    c = (hidden @ attn_w[:, :D].T + attn_b).astype(np.float32)   # [1, H]
    c_cols = np.ascontiguousarray(c.reshape(JT, 128).T)          # [128, JT]
    v_cols = np.ascontiguousarray(v_w.reshape(JT, 128).T).astype(BF16)

    in_maps = []
    for cidx in range(N_CORES):
        sl = enc[cidx * BPC : (cidx + 1) * BPC]                  # [BPC, S, D]
        # (b, t, kk, p) -> (b, p, kk, t)
        encT2 = np.ascontiguousarray(
            sl.reshape(BPC, S, KT, 128).transpose(0, 3, 2, 1).reshape(BPC, 128, KT * S)
        ).astype(BF16)
        in_maps.append(
            {"encT2": encT2, "w_j2": w_j2, "c_cols": c_cols, "v_cols": v_cols}
        )

    trace = bool(os.environ.get("KERNEL_TRACE"))
    if trace:
        _install_prof_shim()
    res = run_bass_kernel_spmd(
        nc, in_maps, core_ids=list(range(N_CORES)), trace=trace
    )
    if trace:
        _cache["last_exec_time_ns"] = res.exec_time_ns
        print(f"HW exec time: {res.exec_time_ns} ns")

    ctx = np.empty((B, 1, D), dtype=np.float32)
    for cidx in range(N_CORES):
        part = np.asarray(res.results[cidx]["out_part"], dtype=np.float32)
        sums = np.asarray(res.results[cidx]["out_sums"], dtype=np.float32)
        for b in range(BPC):
            nch = len(CHUNKS[b])
            acc = part[b][:, : nch * KT].reshape(128, nch, KT).sum(axis=1)
            ctx[cidx * BPC + b, 0, :] = (
                acc / sums[b][0, :nch].sum()
            ).T.reshape(D)
    return ctx


def _install_prof_shim():
    """antenv.axon_hooks is absent from this image; inject it so
    run_bass_kernel_spmd(trace=True) can capture NTFF profiles."""
    import sys
    import types

    if "antenv.axon_hooks" in sys.modules:
        return
    import antenv

    mod = types.ModuleType("antenv.axon_hooks")
    mod._hook = None
    mod.set_axon_ntff_profile_hook = lambda h: setattr(mod, "_hook", h)
    mod.get_axon_ntff_profile_hook = lambda: mod._hook
    sys.modules["antenv.axon_hooks"] = mod
    antenv.axon_hooks = mod
    try:
        from trn_agent_boot.trn_boot import _ntff_profile_via_ctypes

        mod.set_axon_ntff_profile_hook(
            _ntff_profile_via_ctypes("/opt/axon/libaxon_pjrt.so")
        )
    except Exception:
        pass


# revision 33
# speedup vs baseline: 1.0072x; 1.0072x over previous
"""Bass/Trainium2 kernel for nn_Attention_21354577395789.

Reference computation (B=16, S=2048, H=1024, D=2H=2048):
    h      = broadcast(hidden[1, 2H]) -> [B, S, 2H]
    cat    = concat([h, enc], -1)                    [B, S, 4H]
    energy = tanh(cat @ attn_w.T + attn_b)           [B, S, H]
    scores = energy @ v_w.T                          [B, S, 1]
    attn   = softmax(scores, axis=1)
    ctx    = attn^T @ enc                            [B, 1, 2H]

Algebraic simplifications:
  * attn_w = [W_h | W_e] along its 4H input dim, so
    cat @ attn_w.T = hidden @ W_h.T + enc @ W_e.T and
    c = hidden @ W_h.T + attn_b is a single [H] vector shared by every
    (b, s). c is computed on the HOST (it is tiny) and uploaded.
  * scores are O(1) in magnitude, so softmax needs no max subtraction:
    u = exp(s) streamed per 512-chunk of S; the device emits
    per-chunk unnormalized context partials and per-chunk exp-sums,
    and the final (sum over chunks) / (sum of exp) happens on host.

Engine placement per 512-column chunk of S:
  PE    : energy matmuls (the only O(S*D*H) work) + v-reduction
  ACT   : tanh(+bias), exp(+chunk sum)
  GpSimd: broadcast exp-weights row across 128 partitions
  Vector: fused multiply+sum of resident encT tiles against the
          broadcast weights (context partials)
enc is loaded from DRAM exactly once (d-major layout only).

Distribution: data-parallel over B across 8 NeuronCores (2 batches per
core), no collectives. Compute in bf16 (fp32 PSUM accumulation).
"""

import os

import numpy as np
import ml_dtypes

B, S, H = 16, 2048, 1024
D = 2 * H          # 2048, encoder feature dim / contraction dim of W_e
N_CORES = 8
BPC = B // N_CORES  # batches per core = 2
NT = 512           # t-chunk (moving-dim) size
KT = D // 128      # 16 k-tiles over the contraction dim d
JT = H // 128      # 8 j-tiles over the energy dim
TBLK = S // NT     # 4 t-chunks per batch

# Per-batch chunk widths. The LAST batch ends with two narrow chunks so
# the post-energy softmax+context tail (which cannot overlap anything)
# is as short as possible.
CHUNKS = [[512] * 4 for _ in range(BPC)]
CHUNKS[BPC - 1] = [512, 512, 512, 256, 128, 128]
NCH = max(len(c) for c in CHUNKS)

BF16 = ml_dtypes.bfloat16

_cache = {}


def _build():
    import concourse.bacc as bacc
    import concourse.tile as tile
    from concourse import mybir

    nc = bacc.Bacc("TRN2", target_bir_lowering=False, debug=False)
    dt = mybir.dt

    # encT2[b, p, kk*S + t] = enc[b, t, kk*128 + p]
    encT2 = nc.declare_dram_parameter(
        "encT2", [BPC, 128, KT * S], dt.bfloat16, isOutput=False
    )
    # w_j2[p, kk*H + jj*128 + j] = w_eT[kk*128 + p, jj*128 + j]
    # (identical layout to the SBUF-resident copy: one contiguous DMA per kk)
    w_j2 = nc.declare_dram_parameter(
        "w_j2", [128, KT * H], dt.bfloat16, isOutput=False
    )
    c_cols_d = nc.declare_dram_parameter("c_cols", [128, JT], dt.float32, isOutput=False)
    v_cols_d = nc.declare_dram_parameter("v_cols", [128, JT], dt.bfloat16, isOutput=False)
    out_part = nc.declare_dram_parameter(
        "out_part", [BPC, 128, NCH * KT], dt.float32, isOutput=True
    )
    out_sums = nc.declare_dram_parameter(
        "out_sums", [BPC, 1, NCH], dt.float32, isOutput=True
    )

    AF = mybir.ActivationFunctionType
    OP = mybir.AluOpType

    with tile.TileContext(nc) as tc:
        with (
            tc.tile_pool(name="weights", bufs=1) as wpool,
            tc.tile_pool(name="enc", bufs=3) as encpool,
            tc.tile_pool(name="energy", bufs=2) as epool,
            tc.tile_pool(name="usoft", bufs=2) as upool,
            tc.tile_pool(name="scr", bufs=2) as scrpool,
            tc.tile_pool(name="perb", bufs=2) as bpool,
            tc.tile_pool(name="psum_e", bufs=5, space="PSUM") as pe_pool,
            tc.tile_pool(name="psum_s", bufs=2, space="PSUM") as ps_pool,
        ):
            # ---- resident weights/constants -----------------------------
            # kk-major layout (kk*H + jj*128 + j): each per-kk DMA writes
            # one contiguous 2KB run per partition (256B runs fragment DMA
            # into slow small packets), and the jj=0 energy matmuls can
            # start as soon as the first kk slices land.
            w_sb = wpool.tile([128, KT * H], dt.bfloat16, tag="w")
            c_sb = wpool.tile([128, JT], dt.float32, tag="c")
            v_sb = wpool.tile([128, JT], dt.bfloat16, tag="v")

            def w_stat(kk, jj):
                o = kk * H + jj * 128
                return w_sb[:, o : o + 128]

            def dma_w(kk):
                nc.sync.dma_start(
                    w_sb[:, kk * H : (kk + 1) * H],
                    w_j2.ap()[:, kk * H : (kk + 1) * H],
                )

            enc_tiles = {}
            offs = [
                [sum(CHUNKS[b][:ci]) for ci in range(len(CHUNKS[b]))]
                for b in range(BPC)
            ]

            def dma_enc(b, ci, split=False):
                w = CHUNKS[b][ci]
                t0 = offs[b][ci]
                enc_t = encpool.tile(
                    [128, KT * NT], dt.bfloat16, tag="enc", name=f"enc{b}_{ci}"
                )
                src = encT2.ap()[b]
                if split:
                    for k0 in range(0, KT, 4):
                        nc.sync.dma_start(
                            enc_t[:, k0 * w : (k0 + 4) * w].rearrange(
                                "p (kk t) -> p kk t", kk=4, t=w
                            ),
                            src.rearrange("p (kk t) -> p kk t", kk=KT, t=S)[
                                :, k0 : k0 + 4, t0 : t0 + w
                            ],
                        )
                else:
                    nc.sync.dma_start(
                        enc_t[:, : KT * w].rearrange("p (kk t) -> p kk t", kk=KT, t=w),
                        src.rearrange("p (kk t) -> p kk t", kk=KT, t=S)[
                            :, :, t0 : t0 + w
                        ],
                    )
                enc_tiles[(b, ci)] = enc_t

            # startup: interleave per-kk stationary slices with the first
            # enc chunk's 4-kk groups so the jj=0 matmuls stream behind
            # the DMA arrivals kk by kk.
            # constants first: the very first tanh needs c_sb, and these
            # 4.5KB cost nothing in bandwidth
            nc.sync.dma_start(c_sb[:], c_cols_d.ap()[:])
            nc.sync.dma_start(v_sb[:], v_cols_d.ap()[:])
            first = encpool.tile([128, KT * NT], dt.bfloat16, tag="enc", name="enc0_0")
            src0 = encT2.ap()[0].rearrange("p (kk t) -> p kk t", kk=KT, t=S)
            w00 = CHUNKS[0][0]
            for k0 in range(0, KT, 4):
                dma_w(k0)
                dma_w(k0 + 1)
                nc.sync.dma_start(
                    first[:, k0 * w00 : (k0 + 4) * w00].rearrange(
                        "p (kk t) -> p kk t", kk=4, t=w00
                    ),
                    src0[:, k0 : k0 + 4, 0:w00],
                )
                dma_w(k0 + 2)
                dma_w(k0 + 3)
            enc_tiles[(0, 0)] = first
            dma_enc(0, 1)

            sums_t = {}
            part_t = {}
            for b in range(BPC):
                sums_t[b] = bpool.tile(
                    [1, NCH], dt.float32, tag="sums", name=f"sums{b}"
                )
                # per-chunk context partials: part[:, ci*KT + kk]
                part_t[b] = bpool.tile(
                    [128, NCH * KT], dt.float32, tag="part", name=f"part{b}"
                )

            all_chunks = [
                (b, ci) for b in range(BPC) for ci in range(len(CHUNKS[b]))
            ]

            # The v-reduction matmuls of chunk c (which wait on chunk c's
            # tanh outputs) are deferred into chunk c+1's first energy
            # block so the PE never stalls at a chunk boundary, and run as
            # one contiguous block so they break the energy weights'
            # stationary-preload chain only twice per chunk.
            carry = None  # (b, ci, w, s_ps, e_all, enc_t)

            def finish_chunk(b, ci, w, s_ps, enc_t):
                # streaming softmax chunk: u = exp(s), chunk sum
                u_row = upool.tile([1, NT], dt.float16, tag="urow")
                nc.scalar.activation(
                    u_row[:, :w], s_ps[:, :w], AF.Exp,
                    accum_out=sums_t[b][0:1, ci : ci + 1],
                )
                u_bc = upool.tile([128, NT], dt.float16, tag="ubc")
                nc.gpsimd.partition_broadcast(u_bc[:, :w], u_row[:, :w])
                # context partials: part[:, ci*KT+kk] = sum_t u_t * encT[d, t]
                for kk in range(KT):
                    scratch = scrpool.tile([128, NT], dt.bfloat16, tag="scr")
                    nc.vector.scalar_tensor_tensor(
                        out=scratch[:, :w],
                        in0=enc_t[:, kk * w : (kk + 1) * w],
                        scalar=1.0,
                        in1=u_bc[:, :w],
                        op0=OP.mult,
                        op1=OP.mult,
                        accum_out=part_t[b][:, ci * KT + kk : ci * KT + kk + 1],
                    )
                nch = len(CHUNKS[b])
                if ci == nch - 1:
                    nc.sync.dma_start(
                        out_part.ap()[b][:, : nch * KT], part_t[b][:, : nch * KT]
                    )
                    nc.sync.dma_start(
                        out_sums.ap()[b][:, :nch], sums_t[b][0:1, :nch]
                    )

            for b, ci in all_chunks:
                w = CHUNKS[b][ci]
                if (b, ci) not in enc_tiles:
                    dma_enc(b, ci)
                enc_t = enc_tiles.pop((b, ci))
                # prefetch next chunk right away
                nch = len(CHUNKS[b])
                nb, nci = (b, ci + 1) if ci + 1 < nch else (b + 1, 0)
                if nb < BPC and (nb, nci) not in enc_tiles:
                    dma_enc(nb, nci)

                s_ps = ps_pool.tile(
                    [1, NT], dt.float32, tag="sps", name=f"sps{b}_{ci}"
                )
                e_all = epool.tile(
                    [128, JT * NT], dt.bfloat16, tag="eall", name=f"eall{b}_{ci}"
                )
                for jj in range(JT):
                    e_ps = pe_pool.tile([128, NT], dt.float32, tag="eps")
                    for kk in range(KT):
                        nc.tensor.matmul(
                            e_ps[:, :w],
                            w_stat(kk, jj),
                            enc_t[:, kk * w : (kk + 1) * w],
                            start=(kk == 0),
                            stop=(kk == KT - 1),
                        )
                    if jj == 0 and carry is not None:
                        pb, pci, pw, ps_ps, pe_all, penc_t = carry
                        for j in range(JT):
                            nc.tensor.matmul(
                                ps_ps[:, :pw], v_sb[:, j : j + 1],
                                pe_all[:, j * NT : j * NT + pw],
                                start=(j == 0), stop=(j == JT - 1),
                            )
                        finish_chunk(pb, pci, pw, ps_ps, penc_t)
                        carry = None
                    nc.scalar.activation(
                        e_all[:, jj * NT : jj * NT + w], e_ps[:, :w], AF.Tanh,
                        bias=c_sb[:, jj : jj + 1],
                    )
                carry = (b, ci, w, s_ps, e_all, enc_t)

            # drain the final chunk
            pb, pci, pw, ps_ps, pe_all, penc_t = carry
            for j in range(JT):
                nc.tensor.matmul(
                    ps_ps[:, :pw], v_sb[:, j : j + 1],
                    pe_all[:, j * NT : j * NT + pw],
                    start=(j == 0), stop=(j == JT - 1),
                )
            finish_chunk(pb, pci, pw, ps_ps, penc_t)

    nc.compile()
    return nc


def _get_nc():
    if "nc" not in _cache:
        import time

        t0 = time.time()
        _cache["nc"] = _build()
        if os.environ.get("KERNEL_TRACE"):
            print(f"[kernel] bass build+compile: {time.time() - t0:.1f} s")
    return _cache["nc"]


def kernel(hidden, encoder_outputs, attn_w, attn_b, v_w):
    from concourse.bass_utils import run_bass_kernel_spmd

    nc = _get_nc()

    hidden = np.asarray(hidden, dtype=np.float32)
    enc = np.asarray(encoder_outputs, dtype=np.float32)
    attn_w = np.asarray(attn_w, dtype=np.float32)
    attn_b = np.asarray(attn_b, dtype=np.float32)
    v_w = np.asarray(v_w, dtype=np.float32)

    w_eT = np.ascontiguousarray(attn_w[:, D:].T)                 # [D, H]
    # (kk, p, jh) -> (p, kk, jh): same layout as the SBUF-resident copy
    w_j2 = np.ascontiguousarray(
        w_eT.reshape(KT, 128, H).transpose(1, 0, 2).reshape(128, KT * H)
    ).astype(BF16)
    c = (hidden @ attn_w[:, :D].T + attn_b).astype(np.float32)   # [1, H]
    c_cols = np.ascontiguousarray(c.reshape(JT, 128).T)          # [128, JT]
    v_cols = np.ascontiguousarray(v_w.reshape(JT, 128).T).astype(BF16)

    in_maps = []
    for cidx in range(N_CORES):
        sl = enc[cidx * BPC : (cidx + 1) * BPC]                  # [BPC, S, D]
        # (b, t, kk, p) -> (b, p, kk, t)
        encT2 = np.ascontiguousarray(
            sl.reshape(BPC, S, KT, 128).transpose(0, 3, 2, 1).reshape(BPC, 128, KT * S)
        ).astype(BF16)
        in_maps.append(
            {"encT2": encT2, "w_j2": w_j2, "c_cols": c_cols, "v_cols": v_cols}
        )

    trace = bool(os.environ.get("KERNEL_TRACE"))
    if trace:
        _install_prof_shim()
    res = run_bass_kernel_spmd(
        nc, in_maps, core_ids=list(range(N_CORES)), trace=trace
    )
    if trace:
        _cache["last_exec_time_ns"] = res.exec_time_ns
        print(f"HW exec time: {res.exec_time_ns} ns")

    ctx = np.empty((B, 1, D), dtype=np.float32)
    for cidx in range(N_CORES):
        part = np.asarray(res.results[cidx]["out_part"], dtype=np.float32)
        sums = np.asarray(res.results[cidx]["out_sums"], dtype=np.float32)
        for b in range(BPC):
            nch = len(CHUNKS[b])
            acc = part[b][:, : nch * KT].reshape(128, nch, KT).sum(axis=1)
            ctx[cidx * BPC + b, 0, :] = (
                acc / sums[b][0, :nch].sum()
            ).T.reshape(D)
    return ctx


def _install_prof_shim():
    """antenv.axon_hooks is absent from this image; inject it so
    run_bass_kernel_spmd(trace=True) can capture NTFF profiles."""
    import sys
    import types

    if "antenv.axon_hooks" in sys.modules:
        return
    import antenv

    mod = types.ModuleType("antenv.axon_hooks")
    mod._hook = None
    mod.set_axon_ntff_profile_hook = lambda h: setattr(mod, "_hook", h)
    mod.get_axon_ntff_profile_hook = lambda: mod._hook
    sys.modules["antenv.axon_hooks"] = mod
    antenv.axon_hooks = mod
    try:
        from trn_agent_boot.trn_boot import _ntff_profile_via_ctypes

        mod.set_axon_ntff_profile_hook(
            _ntff_profile_via_ctypes("/opt/axon/libaxon_pjrt.so")
        )
    except Exception:
        pass


# revision 34
# speedup vs baseline: 1.0074x; 1.0002x over previous
"""Bass/Trainium2 kernel for nn_Attention_21354577395789.

Reference computation (B=16, S=2048, H=1024, D=2H=2048):
    h      = broadcast(hidden[1, 2H]) -> [B, S, 2H]
    cat    = concat([h, enc], -1)                    [B, S, 4H]
    energy = tanh(cat @ attn_w.T + attn_b)           [B, S, H]
    scores = energy @ v_w.T                          [B, S, 1]
    attn   = softmax(scores, axis=1)
    ctx    = attn^T @ enc                            [B, 1, 2H]

Algebraic simplifications:
  * attn_w = [W_h | W_e] along its 4H input dim, so
    cat @ attn_w.T = hidden @ W_h.T + enc @ W_e.T and
    c = hidden @ W_h.T + attn_b is a single [H] vector shared by every
    (b, s). c is computed on the HOST (it is tiny) and uploaded.
  * scores are O(1) in magnitude, so softmax needs no max subtraction:
    u = exp(s) streamed per 512-chunk of S; the device emits
    per-chunk unnormalized context partials and per-chunk exp-sums,
    and the final (sum over chunks) / (sum of exp) happens on host.

Engine placement per 512-column chunk of S:
  PE    : energy matmuls (the only O(S*D*H) work) + v-reduction
  ACT   : tanh(+bias), exp(+chunk sum)
  GpSimd: broadcast exp-weights row across 128 partitions
  Vector: fused multiply+sum of resident encT tiles against the
          broadcast weights (context partials)
enc is loaded from DRAM exactly once (d-major layout only).

Distribution: data-parallel over B across 8 NeuronCores (2 batches per
core), no collectives. Compute in bf16 (fp32 PSUM accumulation).
"""

import os

import numpy as np
import ml_dtypes

B, S, H = 16, 2048, 1024
D = 2 * H          # 2048, encoder feature dim / contraction dim of W_e
N_CORES = 8
BPC = B // N_CORES  # batches per core = 2
NT = 512           # t-chunk (moving-dim) size
KT = D // 128      # 16 k-tiles over the contraction dim d
JT = H // 128      # 8 j-tiles over the energy dim
TBLK = S // NT     # 4 t-chunks per batch

# Per-batch chunk widths. The LAST batch ends with two narrow chunks so
# the post-energy softmax+context tail (which cannot overlap anything)
# is as short as possible.
CHUNKS = [[512] * 4 for _ in range(BPC)]
CHUNKS[BPC - 1] = [512, 512, 512, 256, 128, 128]
NCH = max(len(c) for c in CHUNKS)

BF16 = ml_dtypes.bfloat16

_cache = {}


def _build():
    import concourse.bacc as bacc
    import concourse.tile as tile
    from concourse import mybir

    nc = bacc.Bacc("TRN2", target_bir_lowering=False, debug=False)
    dt = mybir.dt

    # encT2[b, p, kk*S + t] = enc[b, t, kk*128 + p]
    encT2 = nc.declare_dram_parameter(
        "encT2", [BPC, 128, KT * S], dt.bfloat16, isOutput=False
    )
    # w_j2[p, kk*H + jj*128 + j] = w_eT[kk*128 + p, jj*128 + j]
    # (identical layout to the SBUF-resident copy: one contiguous DMA per kk)
    w_j2 = nc.declare_dram_parameter(
        "w_j2", [128, KT * H], dt.bfloat16, isOutput=False
    )
    c_cols_d = nc.declare_dram_parameter("c_cols", [128, JT], dt.float32, isOutput=False)
    v_cols_d = nc.declare_dram_parameter("v_cols", [128, JT], dt.bfloat16, isOutput=False)
    out_part = nc.declare_dram_parameter(
        "out_part", [BPC, 128, NCH * KT], dt.float32, isOutput=True
    )
    out_sums = nc.declare_dram_parameter(
        "out_sums", [BPC, 1, NCH], dt.float32, isOutput=True
    )

    AF = mybir.ActivationFunctionType
    OP = mybir.AluOpType

    with tile.TileContext(nc) as tc:
        with (
            tc.tile_pool(name="weights", bufs=1) as wpool,
            tc.tile_pool(name="enc", bufs=3) as encpool,
            tc.tile_pool(name="energy", bufs=2) as epool,
            tc.tile_pool(name="perb", bufs=2) as bpool,
            tc.tile_pool(name="psum_e", bufs=5, space="PSUM") as pe_pool,
            tc.tile_pool(name="psum_s", bufs=2, space="PSUM") as ps_pool,
        ):
            # ---- resident weights/constants -----------------------------
            # kk-major layout (kk*H + jj*128 + j): each per-kk DMA writes
            # one contiguous 2KB run per partition (256B runs fragment DMA
            # into slow small packets), and the jj=0 energy matmuls can
            # start as soon as the first kk slices land.
            w_sb = wpool.tile([128, KT * H], dt.bfloat16, tag="w")
            c_sb = wpool.tile([128, JT], dt.float32, tag="c")
            v_sb = wpool.tile([128, JT], dt.bfloat16, tag="v")

            def w_stat(kk, jj):
                o = kk * H + jj * 128
                return w_sb[:, o : o + 128]

            def dma_w(kk):
                nc.sync.dma_start(
                    w_sb[:, kk * H : (kk + 1) * H],
                    w_j2.ap()[:, kk * H : (kk + 1) * H],
                )

            enc_tiles = {}
            offs = [
                [sum(CHUNKS[b][:ci]) for ci in range(len(CHUNKS[b]))]
                for b in range(BPC)
            ]

            def dma_enc(b, ci, split=False):
                w = CHUNKS[b][ci]
                t0 = offs[b][ci]
                enc_t = encpool.tile(
                    [128, KT * NT], dt.bfloat16, tag="enc", name=f"enc{b}_{ci}"
                )
                src = encT2.ap()[b]
                if split:
                    for k0 in range(0, KT, 4):
                        nc.sync.dma_start(
                            enc_t[:, k0 * w : (k0 + 4) * w].rearrange(
                                "p (kk t) -> p kk t", kk=4, t=w
                            ),
                            src.rearrange("p (kk t) -> p kk t", kk=KT, t=S)[
                                :, k0 : k0 + 4, t0 : t0 + w
                            ],
                        )
                else:
                    nc.sync.dma_start(
                        enc_t[:, : KT * w].rearrange("p (kk t) -> p kk t", kk=KT, t=w),
                        src.rearrange("p (kk t) -> p kk t", kk=KT, t=S)[
                            :, :, t0 : t0 + w
                        ],
                    )
                enc_tiles[(b, ci)] = enc_t

            # startup: interleave per-kk stationary slices with the first
            # enc chunk's 4-kk groups so the jj=0 matmuls stream behind
            # the DMA arrivals kk by kk.
            # constants first: the very first tanh needs c_sb, and these
            # 4.5KB cost nothing in bandwidth
            nc.sync.dma_start(c_sb[:], c_cols_d.ap()[:])
            nc.sync.dma_start(v_sb[:], v_cols_d.ap()[:])
            first = encpool.tile([128, KT * NT], dt.bfloat16, tag="enc", name="enc0_0")
            src0 = encT2.ap()[0].rearrange("p (kk t) -> p kk t", kk=KT, t=S)
            w00 = CHUNKS[0][0]
            for k0 in range(0, KT, 4):
                dma_w(k0)
                dma_w(k0 + 1)
                nc.sync.dma_start(
                    first[:, k0 * w00 : (k0 + 4) * w00].rearrange(
                        "p (kk t) -> p kk t", kk=4, t=w00
                    ),
                    src0[:, k0 : k0 + 4, 0:w00],
                )
                dma_w(k0 + 2)
                dma_w(k0 + 3)
            enc_tiles[(0, 0)] = first
            dma_enc(0, 1)

            sums_t = {}
            part_t = {}
            for b in range(BPC):
                sums_t[b] = bpool.tile(
                    [1, NCH], dt.float32, tag="sums", name=f"sums{b}"
                )
                # per-chunk context partials: part[:, ci*KT + kk]
                part_t[b] = bpool.tile(
                    [128, NCH * KT], dt.float32, tag="part", name=f"part{b}"
                )

            all_chunks = [
                (b, ci) for b in range(BPC) for ci in range(len(CHUNKS[b]))
            ]

            # The v-reduction matmuls of chunk c (which wait on chunk c's
            # tanh outputs) are deferred into chunk c+1's first energy
            # block so the PE never stalls at a chunk boundary, and run as
            # one contiguous block so they break the energy weights'
            # stationary-preload chain only twice per chunk.
            carry = None  # (b, ci, w, s_ps, e_all, enc_t)

            def finish_chunk(b, ci, w, s_ps, enc_t):
                # streaming softmax chunk: u = exp(s), chunk sum
                u_row = bpool.tile([1, NT], dt.float16, tag="urow")
                nc.scalar.activation(
                    u_row[:, :w], s_ps[:, :w], AF.Exp,
                    accum_out=sums_t[b][0:1, ci : ci + 1],
                )
                u_bc = bpool.tile([128, NT], dt.float16, tag="ubc")
                nc.gpsimd.partition_broadcast(u_bc[:, :w], u_row[:, :w])
                # context partials: part[:, ci*KT+kk] = sum_t u_t * encT[d, t]
                for kk in range(KT):
                    scratch = bpool.tile([128, NT], dt.bfloat16, tag="scr")
                    nc.vector.scalar_tensor_tensor(
                        out=scratch[:, :w],
                        in0=enc_t[:, kk * w : (kk + 1) * w],
                        scalar=1.0,
                        in1=u_bc[:, :w],
                        op0=OP.mult,
                        op1=OP.mult,
                        accum_out=part_t[b][:, ci * KT + kk : ci * KT + kk + 1],
                    )
                nc.sync.dma_start(
                    out_part.ap()[b][:, ci * KT : (ci + 1) * KT],
                    part_t[b][:, ci * KT : (ci + 1) * KT],
                )
                nch = len(CHUNKS[b])
                if ci == nch - 1:
                    nc.sync.dma_start(
                        out_sums.ap()[b][:, :nch], sums_t[b][0:1, :nch]
                    )

            for b, ci in all_chunks:
                w = CHUNKS[b][ci]
                if (b, ci) not in enc_tiles:
                    dma_enc(b, ci)
                enc_t = enc_tiles.pop((b, ci))
                # prefetch next chunk right away
                nch = len(CHUNKS[b])
                nb, nci = (b, ci + 1) if ci + 1 < nch else (b + 1, 0)
                if nb < BPC and (nb, nci) not in enc_tiles:
                    dma_enc(nb, nci)

                s_ps = ps_pool.tile(
                    [1, NT], dt.float32, tag="sps", name=f"sps{b}_{ci}"
                )
                e_all = epool.tile(
                    [128, JT * NT], dt.bfloat16, tag="eall", name=f"eall{b}_{ci}"
                )
                for jj in range(JT):
                    e_ps = pe_pool.tile([128, NT], dt.float32, tag="eps")
                    for kk in range(KT):
                        nc.tensor.matmul(
                            e_ps[:, :w],
                            w_stat(kk, jj),
                            enc_t[:, kk * w : (kk + 1) * w],
                            start=(kk == 0),
                            stop=(kk == KT - 1),
                        )
                    if jj == 0 and carry is not None:
                        pb, pci, pw, ps_ps, pe_all, penc_t = carry
                        for j in range(JT):
                            nc.tensor.matmul(
                                ps_ps[:, :pw], v_sb[:, j : j + 1],
                                pe_all[:, j * NT : j * NT + pw],
                                start=(j == 0), stop=(j == JT - 1),
                            )
                        finish_chunk(pb, pci, pw, ps_ps, penc_t)
                        carry = None
                    nc.scalar.activation(
                        e_all[:, jj * NT : jj * NT + w], e_ps[:, :w], AF.Tanh,
                        bias=c_sb[:, jj : jj + 1],
                    )
                carry = (b, ci, w, s_ps, e_all, enc_t)

            # drain the final chunk
            pb, pci, pw, ps_ps, pe_all, penc_t = carry
            for j in range(JT):
                nc.tensor.matmul(
                    ps_ps[:, :pw], v_sb[:, j : j + 1],
                    pe_all[:, j * NT : j * NT + pw],
                    start=(j == 0), stop=(j == JT - 1),
                )
            finish_chunk(pb, pci, pw, ps_ps, penc_t)

    nc.compile()
    return nc


def _get_nc():
    if "nc" not in _cache:
        import time

        t0 = time.time()
        _cache["nc"] = _build()
        if os.environ.get("KERNEL_TRACE"):
            print(f"[kernel] bass build+compile: {time.time() - t0:.1f} s")
    return _cache["nc"]


def kernel(hidden, encoder_outputs, attn_w, attn_b, v_w):
    from concourse.bass_utils import run_bass_kernel_spmd

    nc = _get_nc()

    hidden = np.asarray(hidden, dtype=np.float32)
    enc = np.asarray(encoder_outputs, dtype=np.float32)
    attn_w = np.asarray(attn_w, dtype=np.float32)
    attn_b = np.asarray(attn_b, dtype=np.float32)
    v_w = np.asarray(v_w, dtype=np.float32)

    w_eT = np.ascontiguousarray(attn_w[:, D:].T)                 # [D, H]
    # (kk, p, jh) -> (p, kk, jh): same layout as the SBUF-resident copy
    w_j2 = np.ascontiguousarray(
        w_eT.reshape(KT, 128, H).transpose(1, 0, 2).reshape(128, KT * H)
    ).astype(BF16)
    c = (hidden @ attn_w[:, :D].T + attn_b).astype(np.float32)   # [1, H]
    c_cols = np.ascontiguousarray(c.reshape(JT, 128).T)          # [128, JT]
    v_cols = np.ascontiguousarray(v_w.reshape(JT, 128).T).astype(BF16)

    in_maps = []
    for cidx in range(N_CORES):
        sl = enc[cidx * BPC : (cidx + 1) * BPC]                  # [BPC, S, D]
        # (b, t, kk, p) -> (b, p, kk, t)
        encT2 = np.ascontiguousarray(
            sl.reshape(BPC, S, KT, 128).transpose(0, 3, 2, 1).reshape(BPC, 128, KT * S)
        ).astype(BF16)
        in_maps.append(
            {"encT2": encT2, "w_j2": w_j2, "c_cols": c_cols, "v_cols": v_cols}
        )

    trace = bool(os.environ.get("KERNEL_TRACE"))
    if trace:
        _install_prof_shim()
    res = run_bass_kernel_spmd(
        nc, in_maps, core_ids=list(range(N_CORES)), trace=trace
    )
    if trace:
        _cache["last_exec_time_ns"] = res.exec_time_ns
        print(f"HW exec time: {res.exec_time_ns} ns")

    ctx = np.empty((B, 1, D), dtype=np.float32)
    for cidx in range(N_CORES):
        part = np.asarray(res.results[cidx]["out_part"], dtype=np.float32)
        sums = np.asarray(res.results[cidx]["out_sums"], dtype=np.float32)
        for b in range(BPC):
            nch = len(CHUNKS[b])
            acc = part[b][:, : nch * KT].reshape(128, nch, KT).sum(axis=1)
            ctx[cidx * BPC + b, 0, :] = (
                acc / sums[b][0, :nch].sum()
            ).T.reshape(D)
    return ctx


def _install_prof_shim():
    """antenv.axon_hooks is absent from this image; inject it so
    run_bass_kernel_spmd(trace=True) can capture NTFF profiles."""
    import sys
    import types

    if "antenv.axon_hooks" in sys.modules:
        return
    import antenv

    mod = types.ModuleType("antenv.axon_hooks")
    mod._hook = None
    mod.set_axon_ntff_profile_hook = lambda h: setattr(mod, "_hook", h)
    mod.get_axon_ntff_profile_hook = lambda: mod._hook
    sys.modules["antenv.axon_hooks"] = mod
    antenv.axon_hooks = mod
    try:
        from trn_agent_boot.trn_boot import _ntff_profile_via_ctypes

        mod.set_axon_ntff_profile_hook(
            _ntff_profile_via_ctypes("/opt/axon/libaxon_pjrt.so")
        )
    except Exception:
        pass


# revision 37
# speedup vs baseline: 1.0367x; 1.0292x over previous
"""Bass/Trainium2 kernel for nn_Attention_21354577395789.

Reference computation (B=16, S=2048, H=1024, D=2H=2048):
    h      = broadcast(hidden[1, 2H]) -> [B, S, 2H]
    cat    = concat([h, enc], -1)                    [B, S, 4H]
    energy = tanh(cat @ attn_w.T + attn_b)           [B, S, H]
    scores = energy @ v_w.T                          [B, S, 1]
    attn   = softmax(scores, axis=1)
    ctx    = attn^T @ enc                            [B, 1, 2H]

Algebraic simplifications:
  * attn_w = [W_h | W_e] along its 4H input dim, so
    cat @ attn_w.T = hidden @ W_h.T + enc @ W_e.T and
    c = hidden @ W_h.T + attn_b is a single [H] vector shared by every
    (b, s). c is computed on the HOST (it is tiny) and uploaded.
  * scores are O(1) in magnitude, so softmax needs no max subtraction:
    u = exp(s) streamed per 512-chunk of S; the device emits
    per-chunk unnormalized context partials and per-chunk exp-sums,
    and the final (sum over chunks) / (sum of exp) happens on host.

Engine placement per 512-column chunk of S:
  PE    : energy matmuls (the only O(S*D*H) work) + v-reduction
  ACT   : tanh(+bias), exp(+chunk sum)
  GpSimd: broadcast exp-weights row across 128 partitions
  Vector: fused multiply+sum of resident encT tiles against the
          broadcast weights (context partials)
enc is loaded from DRAM exactly once (d-major layout only).

Distribution: data-parallel over B across 8 NeuronCores (2 batches per
core), no collectives. Compute in bf16 (fp32 PSUM accumulation).
"""

import os

import numpy as np
import ml_dtypes

B, S, H = 16, 2048, 1024
D = 2 * H          # 2048, encoder feature dim / contraction dim of W_e
N_CORES = 8
BPC = B // N_CORES  # batches per core = 2
NT = 512           # t-chunk (moving-dim) size
KT = D // 128      # 16 k-tiles over the contraction dim d
JT = H // 128      # 8 j-tiles over the energy dim
TBLK = S // NT     # 4 t-chunks per batch

# Per-batch chunk widths. The LAST batch ends with two narrow chunks so
# the post-energy softmax+context tail (which cannot overlap anything)
# is as short as possible.
CHUNKS = [[512] * 4 for _ in range(BPC)]
CHUNKS[BPC - 1] = [512, 512, 512, 256, 128, 128]
NCH = max(len(c) for c in CHUNKS)

BF16 = ml_dtypes.bfloat16

_cache = {}


def _build():
    import concourse.bacc as bacc
    import concourse.tile as tile
    from concourse import mybir

    nc = bacc.Bacc("TRN2", target_bir_lowering=False, debug=False)
    dt = mybir.dt

    # encT2[b, p, kk*S + t] = enc[b, t, kk*128 + p]
    encT2 = nc.declare_dram_parameter(
        "encT2", [BPC, 128, KT * S], dt.bfloat16, isOutput=False
    )
    # w_j2[p, kk*H + jj*128 + j] = w_eT[kk*128 + p, jj*128 + j]
    # (identical layout to the SBUF-resident copy: one contiguous DMA per kk)
    w_j2 = nc.declare_dram_parameter(
        "w_j2", [128, KT * H], dt.bfloat16, isOutput=False
    )
    c_cols_d = nc.declare_dram_parameter("c_cols", [128, JT], dt.float32, isOutput=False)
    v_cols_d = nc.declare_dram_parameter("v_cols", [128, JT], dt.float32, isOutput=False)
    out_part = nc.declare_dram_parameter(
        "out_part", [BPC, 128, NCH * KT], dt.float32, isOutput=True
    )
    out_sums = nc.declare_dram_parameter(
        "out_sums", [BPC, 1, NCH], dt.float32, isOutput=True
    )

    AF = mybir.ActivationFunctionType
    OP = mybir.AluOpType

    with tile.TileContext(nc) as tc:
        with (
            tc.tile_pool(name="weights", bufs=1) as wpool,
            tc.tile_pool(name="enc", bufs=3) as encpool,
            tc.tile_pool(name="energy", bufs=2) as epool,
            tc.tile_pool(name="perb", bufs=2) as bpool,
            tc.tile_pool(name="psum_e", bufs=5, space="PSUM") as pe_pool,
            tc.tile_pool(name="psum_s", bufs=2, space="PSUM") as ps_pool,
        ):
            # ---- resident weights/constants -----------------------------
            # kk-major layout (kk*H + jj*128 + j): each per-kk DMA writes
            # one contiguous 2KB run per partition (256B runs fragment DMA
            # into slow small packets), and the jj=0 energy matmuls can
            # start as soon as the first kk slices land.
            w_sb = wpool.tile([128, KT * H], dt.bfloat16, tag="w")
            c_sb = wpool.tile([128, JT], dt.float32, tag="c")
            v_sb = wpool.tile([128, JT], dt.float32, tag="v")

            def w_stat(kk, jj):
                o = kk * H + jj * 128
                return w_sb[:, o : o + 128]

            def dma_w(kk):
                nc.sync.dma_start(
                    w_sb[:, kk * H : (kk + 1) * H],
                    w_j2.ap()[:, kk * H : (kk + 1) * H],
                )

            enc_tiles = {}
            offs = [
                [sum(CHUNKS[b][:ci]) for ci in range(len(CHUNKS[b]))]
                for b in range(BPC)
            ]

            def dma_enc(b, ci, split=False):
                w = CHUNKS[b][ci]
                t0 = offs[b][ci]
                enc_t = encpool.tile(
                    [128, KT * NT], dt.bfloat16, tag="enc", name=f"enc{b}_{ci}"
                )
                src = encT2.ap()[b]
                if split:
                    for k0 in range(0, KT, 4):
                        nc.sync.dma_start(
                            enc_t[:, k0 * w : (k0 + 4) * w].rearrange(
                                "p (kk t) -> p kk t", kk=4, t=w
                            ),
                            src.rearrange("p (kk t) -> p kk t", kk=KT, t=S)[
                                :, k0 : k0 + 4, t0 : t0 + w
                            ],
                        )
                else:
                    nc.sync.dma_start(
                        enc_t[:, : KT * w].rearrange("p (kk t) -> p kk t", kk=KT, t=w),
                        src.rearrange("p (kk t) -> p kk t", kk=KT, t=S)[
                            :, :, t0 : t0 + w
                        ],
                    )
                enc_tiles[(b, ci)] = enc_t

            # startup: interleave per-kk stationary slices with the first
            # enc chunk's 4-kk groups so the jj=0 matmuls stream behind
            # the DMA arrivals kk by kk.
            # constants first: the very first tanh needs c_sb, and these
            # 4.5KB cost nothing in bandwidth
            nc.sync.dma_start(c_sb[:], c_cols_d.ap()[:])
            nc.sync.dma_start(v_sb[:], v_cols_d.ap()[:])
            first = encpool.tile([128, KT * NT], dt.bfloat16, tag="enc", name="enc0_0")
            src0 = encT2.ap()[0].rearrange("p (kk t) -> p kk t", kk=KT, t=S)
            w00 = CHUNKS[0][0]
            for k0 in range(0, KT, 4):
                dma_w(k0)
                dma_w(k0 + 1)
                nc.sync.dma_start(
                    first[:, k0 * w00 : (k0 + 4) * w00].rearrange(
                        "p (kk t) -> p kk t", kk=4, t=w00
                    ),
                    src0[:, k0 : k0 + 4, 0:w00],
                )
                dma_w(k0 + 2)
                dma_w(k0 + 3)
            enc_tiles[(0, 0)] = first
            dma_enc(0, 1)

            sums_t = {}
            part_t = {}
            for b in range(BPC):
                sums_t[b] = bpool.tile(
                    [1, NCH], dt.float32, tag="sums", name=f"sums{b}"
                )
                # per-chunk context partials: part[:, ci*KT + kk]
                part_t[b] = bpool.tile(
                    [128, NCH * KT], dt.float32, tag="part", name=f"part{b}"
                )

            all_chunks = [
                (b, ci) for b in range(BPC) for ci in range(len(CHUNKS[b]))
            ]

            # Scores: the Vector engine pre-multiplies each tanh tile by
            # its v segment (tensor_scalar, 4x mode) and tree-adds the 8
            # products in fp16, so the PE does a SINGLE ones-stationary
            # partition-reduce matmul per chunk instead of 8 v-stationary
            # ones (saves 7/8 of the v-reduction matmul columns). That
            # matmul is deferred into chunk c+1's first energy block so
            # the PE never stalls at a chunk boundary.
            ones_col = wpool.tile([128, 1], dt.float16, tag="ones")
            nc.vector.memset(ones_col[:], 1.0)
            carry = None  # (b, ci, w, s_ps, esum, enc_t)

            def finish_chunk(b, ci, w, s_ps, enc_t):
                # streaming softmax chunk: u = exp(s), chunk sum
                u_row = bpool.tile([1, NT], dt.float16, tag="urow")
                nc.scalar.activation(
                    u_row[:, :w], s_ps[:, :w], AF.Exp,
                    accum_out=sums_t[b][0:1, ci : ci + 1],
                )
                u_bc = bpool.tile([128, NT], dt.float16, tag="ubc")
                nc.gpsimd.partition_broadcast(u_bc[:, :w], u_row[:, :w])
                # context partials: part[:, ci*KT+kk] = sum_t u_t * encT[d, t]
                for kk in range(KT):
                    scratch = bpool.tile([128, NT], dt.bfloat16, tag="scr")
                    nc.vector.scalar_tensor_tensor(
                        out=scratch[:, :w],
                        in0=enc_t[:, kk * w : (kk + 1) * w],
                        scalar=1.0,
                        in1=u_bc[:, :w],
                        op0=OP.mult,
                        op1=OP.mult,
                        accum_out=part_t[b][:, ci * KT + kk : ci * KT + kk + 1],
                    )
                nc.sync.dma_start(
                    out_part.ap()[b][:, ci * KT : (ci + 1) * KT],
                    part_t[b][:, ci * KT : (ci + 1) * KT],
                )
                nch = len(CHUNKS[b])
                if ci == nch - 1:
                    nc.sync.dma_start(
                        out_sums.ap()[b][:, :nch], sums_t[b][0:1, :nch]
                    )

            for b, ci in all_chunks:
                w = CHUNKS[b][ci]
                if (b, ci) not in enc_tiles:
                    dma_enc(b, ci)
                enc_t = enc_tiles.pop((b, ci))
                # prefetch next chunk right away
                nch = len(CHUNKS[b])
                nb, nci = (b, ci + 1) if ci + 1 < nch else (b + 1, 0)
                if nb < BPC and (nb, nci) not in enc_tiles:
                    dma_enc(nb, nci)

                s_ps = ps_pool.tile(
                    [1, NT], dt.float32, tag="sps", name=f"sps{b}_{ci}"
                )
                e_all = epool.tile(
                    [128, JT * NT], dt.bfloat16, tag="eall", name=f"eall{b}_{ci}"
                )
                acc = None
                for jj in range(JT):
                    e_ps = pe_pool.tile([128, NT], dt.float32, tag="eps")
                    for kk in range(KT):
                        nc.tensor.matmul(
                            e_ps[:, :w],
                            w_stat(kk, jj),
                            enc_t[:, kk * w : (kk + 1) * w],
                            start=(kk == 0),
                            stop=(kk == KT - 1),
                        )
                    if jj == 0 and carry is not None:
                        pb, pci, pw, ps_ps, pesum, penc_t = carry
                        nc.tensor.matmul(
                            ps_ps[:, :pw], ones_col[:], pesum[:, :pw],
                            start=True, stop=True,
                        )
                        finish_chunk(pb, pci, pw, ps_ps, penc_t)
                        carry = None
                    nc.scalar.activation(
                        e_all[:, jj * NT : jj * NT + w], e_ps[:, :w], AF.Tanh,
                        bias=c_sb[:, jj : jj + 1],
                    )
                    ev = bpool.tile(
                        [128, NT], dt.float16, tag="ev", bufs=3, name=f"ev{jj}"
                    )
                    nc.vector.tensor_scalar_mul(
                        ev[:, :w], e_all[:, jj * NT : jj * NT + w],
                        v_sb[:, jj : jj + 1],
                    )
                    if acc is None:
                        acc = ev
                    else:
                        nacc = bpool.tile(
                            [128, NT], dt.float16, tag="esum", bufs=3,
                            name=f"esum{jj}",
                        )
                        nc.vector.tensor_add(nacc[:, :w], acc[:, :w], ev[:, :w])
                        acc = nacc
                carry = (b, ci, w, s_ps, acc, enc_t)

            # drain the final chunk
            pb, pci, pw, ps_ps, pesum, penc_t = carry
            nc.tensor.matmul(
                ps_ps[:, :pw], ones_col[:], pesum[:, :pw], start=True, stop=True
            )
            finish_chunk(pb, pci, pw, ps_ps, penc_t)

    nc.compile()
    return nc


def _get_nc():
    if "nc" not in _cache:
        import time

        t0 = time.time()
        _cache["nc"] = _build()
        if os.environ.get("KERNEL_TRACE"):
            print(f"[kernel] bass build+compile: {time.time() - t0:.1f} s")
    return _cache["nc"]


def kernel(hidden, encoder_outputs, attn_w, attn_b, v_w):
    from concourse.bass_utils import run_bass_kernel_spmd

    nc = _get_nc()

    hidden = np.asarray(hidden, dtype=np.float32)
    enc = np.asarray(encoder_outputs, dtype=np.float32)
    attn_w = np.asarray(attn_w, dtype=np.float32)
    attn_b = np.asarray(attn_b, dtype=np.float32)
    v_w = np.asarray(v_w, dtype=np.float32)

    w_eT = np.ascontiguousarray(attn_w[:, D:].T)                 # [D, H]
    # (kk, p, jh) -> (p, kk, jh): same layout as the SBUF-resident copy
    w_j2 = np.ascontiguousarray(
        w_eT.reshape(KT, 128, H).transpose(1, 0, 2).reshape(128, KT * H)
    ).astype(BF16)
    c = (hidden @ attn_w[:, :D].T + attn_b).astype(np.float32)   # [1, H]
    c_cols = np.ascontiguousarray(c.reshape(JT, 128).T)          # [128, JT]
    v_cols = np.ascontiguousarray(v_w.reshape(JT, 128).T)

    in_maps = []
    for cidx in range(N_CORES):
        sl = enc[cidx * BPC : (cidx + 1) * BPC]                  # [BPC, S, D]
        # (b, t, kk, p) -> (b, p, kk, t)
        encT2 = np.ascontiguousarray(
            sl.reshape(BPC, S, KT, 128).transpose(0, 3, 2, 1).reshape(BPC, 128, KT * S)
        ).astype(BF16)
        in_maps.append(
            {"encT2": encT2, "w_j2": w_j2, "c_cols": c_cols, "v_cols": v_cols}
        )

    trace = bool(os.environ.get("KERNEL_TRACE"))
    if trace:
        _install_prof_shim()
    res = run_bass_kernel_spmd(
        nc, in_maps, core_ids=list(range(N_CORES)), trace=trace
    )
    if trace:
        _cache["last_exec_time_ns"] = res.exec_time_ns
        print(f"HW exec time: {res.exec_time_ns} ns")

    ctx = np.empty((B, 1, D), dtype=np.float32)
    for cidx in range(N_CORES):
        part = np.asarray(res.results[cidx]["out_part"], dtype=np.float32)
        sums = np.asarray(res.results[cidx]["out_sums"], dtype=np.float32)
        for b in range(BPC):
            nch = len(CHUNKS[b])
            acc = part[b][:, : nch * KT].reshape(128, nch, KT).sum(axis=1)
            ctx[cidx * BPC + b, 0, :] = (
                acc / sums[b][0, :nch].sum()
            ).T.reshape(D)
    return ctx


def _install_prof_shim():
    """antenv.axon_hooks is absent from this image; inject it so
    run_bass_kernel_spmd(trace=True) can capture NTFF profiles."""
    import sys
    import types

    if "antenv.axon_hooks" in sys.modules:
        return
    import antenv

    mod = types.ModuleType("antenv.axon_hooks")
    mod._hook = None
    mod.set_axon_ntff_profile_hook = lambda h: setattr(mod, "_hook", h)
    mod.get_axon_ntff_profile_hook = lambda: mod._hook
    sys.modules["antenv.axon_hooks"] = mod
    antenv.axon_hooks = mod
    try:
        from trn_agent_boot.trn_boot import _ntff_profile_via_ctypes

        mod.set_axon_ntff_profile_hook(
            _ntff_profile_via_ctypes("/opt/axon/libaxon_pjrt.so")
        )
    except Exception:
        pass
